# revision 1
# baseline (speedup 1.0000x reference)
"""3-hop GNN message passing (BPR/LightGCN style) on 8 Trainium2 NeuronCores.

Strategy: destination-sharded SpMMs. Each of the 5 segment-sum SpMMs is
computed by sharding edges across cores by destination row; each core
gathers source rows with bulk dma_gather (int16 indices, 25000-row table
sections), applies per-edge weights via one-hot matmuls on the tensor
engine (S[e, slot] = w_e, slot = dest - block_base), accumulates per-block
PSUM windows, and scatter-adds the finished rows to HBM. AllGathers
assemble full intermediate tables between hops. The weighted residual
combine runs on-device; the host only sorts/buckets edges and concatenates
the 8 per-core output slices.
"""
import sys
import os

sys.path.insert(0, "/opt/trn_rl_repo")

import numpy as np

import concourse.bass as bass
import concourse.bacc as bacc
import concourse.tile as tile
from concourse import bass_utils, mybir

# problem constants (hardcoded per harness contract)
U, I, D, E = 100000, 50000, 64, 1250000
NCORES = 8
DU = U // NCORES           # users per core (dest shard for ui-SpMMs)
DI = I // NCORES           # items per core (dest shard for iu-SpMMs)
SEC = 25000                # table section rows (int16 gather index range)
NSEC_IU = U // SEC         # 4 sections of the user-side tables
NSEC_UI = I // SEC         # 2 sections of the item-side tables
W = 64                     # dest window (one-hot slot count)
K = 128                    # edges per chunk (PE contraction dim)
CPB = 3                    # chunks per block
BPS = 16                   # blocks per super-block (= 2 PSUM banks)
CH_SB = BPS * CPB          # 48 chunks per super-block
IDX_SB = CH_SB * K         # 6144 gather indices per super-block
ROWS_SB = BPS * W          # 1024 scatter rows per super-block

_LAST_RESULTS = None       # run metadata for test harness


def _pack_type(dest, src, w, dshard, nsec):
    """Pack edges for one SpMM type (iu or ui) into the uniform SPMD layout.

    Returns dict with per-core arrays (idx16, slot, w, sidx16) and NSB.
    """
    dest = dest.astype(np.int64)
    src = src.astype(np.int64)
    w = w.astype(np.float32)
    core_of = dest // dshard
    sec_of = src // SEC

    # per (core, section): lists of (slot_stream, src_stream, w_stream, blocks)
    per_cs = {}
    nblk_max = 0
    for c in range(NCORES):
        for s in range(nsec):
            m = (core_of == c) & (sec_of == s)
            d = dest[m] - c * dshard
            sl = src[m] - s * SEC
            wv = w[m]
            order = np.argsort(d, kind="stable")
            d, sl, wv = d[order], sl[order], wv[order]
            # unique dests in order with counts
            ud, ustart, ucnt = np.unique(d, return_index=True, return_counts=True)
            blocks = []   # (base, span, nedges)
            cur_base = -1
            cur_cnt = 0
            cur_edges = []  # list of (start, count, slotbase)
            slot_arr = np.empty(len(d), np.float32)
            blk_of_edge = np.empty(len(d), np.int64)
            cap = CPB * K
            for t in range(len(ud)):
                u, st, k = int(ud[t]), int(ustart[t]), int(ucnt[t])
                if cur_base < 0 or (u - cur_base) >= W or (cur_cnt + k) > cap:
                    if cur_base >= 0:
                        blocks.append((cur_base, cur_span, cur_cnt))
                    cur_base = u
                    cur_cnt = 0
                cur_span = u - cur_base + 1
                slot_arr[st:st + k] = u - cur_base
                blk_of_edge[st:st + k] = len(blocks)
                cur_cnt += k
            if cur_base >= 0:
                blocks.append((cur_base, cur_span, cur_cnt))
            per_cs[(c, s)] = (d, sl, wv, slot_arr, blk_of_edge, blocks)
            nblk_max = max(nblk_max, len(blocks))

    nsb = (nblk_max + BPS - 1) // BPS
    nblk = nsb * BPS

    # emit per-core uniform arrays
    ncols_ch = nsec * nsb * CH_SB          # chunk columns total
    out = {
        "idx16": np.zeros((NCORES, 128, nsec * nsb * IDX_SB // 16), np.int16),
        "slot": np.zeros((NCORES, 128, ncols_ch), np.float32),
        "w": np.zeros((NCORES, 128, ncols_ch), np.float32),
        "sidx16": np.zeros((NCORES, 128, nsec * nsb * ROWS_SB // 16), np.int16),
        "nsb": nsb,
    }
    trash = dshard  # rows [dshard, dshard+W) are trash
    for c in range(NCORES):
        for s in range(nsec):
            d, sl, wv, slot_arr, blk_of_edge, blocks = per_cs[(c, s)]
            nb = len(blocks)
            # stream arrays padded to nblk blocks
            slots_total = nblk * CPB * K
            idx_st = np.zeros(slots_total, np.int16)
            slot_st = np.zeros(slots_total, np.float32)
            w_st = np.zeros(slots_total, np.float32)
            # place each block's edges at block*cap
            edge_pos_in_blk = np.zeros(len(d), np.int64)
            # cumulative position within block
            if len(d):
                # edges are already grouped by block in order
                blk_change = np.r_[True, blk_of_edge[1:] != blk_of_edge[:-1]]
                grp_start = np.maximum.accumulate(np.where(blk_change, np.arange(len(d)), 0))
                edge_pos_in_blk = np.arange(len(d)) - grp_start
                pos = blk_of_edge * (CPB * K) + edge_pos_in_blk
                idx_st[pos] = sl.astype(np.int16)
                slot_st[pos] = slot_arr
                w_st[pos] = wv
            # wrap into device layouts
            base_col = s * nsb  # super-block offset for this section
            # gather idx: position i -> (row i%16, col i//16), tiled 8x
            idxw = idx_st.reshape(-1, 16).T  # [16, slots/16]
            cw0 = base_col * (IDX_SB // 16)
            out["idx16"][c][:, cw0:cw0 + idxw.shape[1]] = np.tile(idxw, (8, 1))
            # slot/w: chunk-major [128, cols]
            sm = slot_st.reshape(-1, K).T    # [128, ncols_cs]
            wm = w_st.reshape(-1, K).T
            cc0 = base_col * CH_SB
            out["slot"][c][:, cc0:cc0 + sm.shape[1]] = sm
            out["w"][c][:, cc0:cc0 + wm.shape[1]] = wm
            # scatter rows: per super-block 1024 rows; row n -> p=n%128, j=n//128
            # p<64: block 8*sb_local... (block = j + 8*(p>=64), slot r)
            srows = np.full(nblk * W, trash, np.int64)
            rr = np.arange(nblk * W)
            srows += rr % W  # default trash + r (unique per slot)
            for b, (base, span, cnt) in enumerate(blocks):
                r = np.arange(span)
                srows[b * W + r[:span]] = base + r[:span]
                # rows span..W-1 remain trash + r
            # reorder into scatter enumeration: for each sb: n in [0,1024):
            # p = n%128, j = n//128; block_local = j + 8*(p>=64); r = p%64
            sidx_strm = np.empty(nblk * W, np.int16)
            n = np.arange(nsb * ROWS_SB)
            p = n % 128
            j = (n // 128) % 8
            sb_i = n // ROWS_SB
            blk_l = sb_i * BPS + j + 8 * (p >= 64)
            r = p % 64
            sidx_strm = srows[blk_l * W + r].astype(np.int16)
            sw = sidx_strm.reshape(-1, 16).T
            sc0 = base_col * (ROWS_SB // 16)
            out["sidx16"][c][:, sc0:sc0 + sw.shape[1]] = np.tile(sw, (8, 1))
    return out


def _build_program(nsb_iu, nsb_ui):
    nc = bacc.Bacc("TRN2", target_bir_lowering=False, debug=False,
                   num_devices=NCORES)
    f32 = mybir.dt.float32
    i16 = mybir.dt.int16

    t_eu = nc.dram_tensor("embed_user", [U, D], f32, kind="ExternalInput")
    t_ei = nc.dram_tensor("embed_item", [I, D], f32, kind="ExternalInput")
    ei_slice = nc.dram_tensor("ei_slice", [DI, D], f32, kind="ExternalInput")
    iota_in = nc.dram_tensor("iota", [128, W], f32, kind="ExternalInput")

    iu_cols = NSEC_IU * nsb_iu
    ui_cols = NSEC_UI * nsb_ui
    iu_idx = nc.dram_tensor("iu_idx", [128, iu_cols * IDX_SB // 16], i16, kind="ExternalInput")
    iu_slot = nc.dram_tensor("iu_slot", [128, iu_cols * CH_SB], f32, kind="ExternalInput")
    iu_w = nc.dram_tensor("iu_w", [128, iu_cols * CH_SB], f32, kind="ExternalInput")
    iu_sidx = nc.dram_tensor("iu_sidx", [128, iu_cols * ROWS_SB // 16], i16, kind="ExternalInput")
    ui_idx = nc.dram_tensor("ui_idx", [128, ui_cols * IDX_SB // 16], i16, kind="ExternalInput")
    ui_slot = nc.dram_tensor("ui_slot", [128, ui_cols * CH_SB], f32, kind="ExternalInput")
    ui_w = nc.dram_tensor("ui_w", [128, ui_cols * CH_SB], f32, kind="ExternalInput")
    ui_sidx = nc.dram_tensor("ui_sidx", [128, ui_cols * ROWS_SB // 16], i16, kind="ExternalInput")

    out_ext = nc.dram_tensor("out", [DI, D], f32, kind="ExternalOutput")

    g1i_part = nc.dram_tensor("g1i_part", [DI + W, D], f32, kind="Internal")
    g1u_part = nc.dram_tensor("g1u_part", [DU + W, D], f32, kind="Internal")
    g2u_part = nc.dram_tensor("g2u_part", [DU + W, D], f32, kind="Internal")
    g2i_part = nc.dram_tensor("g2i_part", [DI + W, D], f32, kind="Internal")
    g3i_part = nc.dram_tensor("g3i_part", [DI + W, D], f32, kind="Internal")
    g1i_full = nc.dram_tensor("g1i_full", [I, D], f32, kind="Internal")
    g1u_full = nc.dram_tensor("g1u_full", [U, D], f32, kind="Internal")
    g2u_full = nc.dram_tensor("g2u_full", [U, D], f32, kind="Internal")

    rg = [list(range(NCORES))]

    with tile.TileContext(nc) as tc:
        with (
            tc.tile_pool(name="const", bufs=1) as cpool,
            tc.tile_pool(name="sb", bufs=2) as sb,
            tc.tile_pool(name="gp", bufs=3) as gp,
            tc.tile_pool(name="spool", bufs=6) as spool,
            tc.tile_pool(name="psum", bufs=4, space="PSUM") as pp,
        ):
            iota_t = cpool.tile([128, W], f32)
            nc.sync.dma_start(out=iota_t[:], in_=iota_in[:])

            # zero tile for clearing part tensors
            zt = cpool.tile([128, 48 * D], f32)
            nc.vector.memset(zt[:], 0.0)

            def zero_part(part, nrows):
                full = (nrows // (128 * 48)) * 128 * 48
                r0 = 0
                step = 128 * 48
                while r0 < nrows:
                    n = min(step, nrows - r0)
                    a = n // 128
                    if a >= 1:
                        nc.sync.dma_start(
                            out=part[r0:r0 + a * 128, :].rearrange(
                                "(a p) d -> p a d", p=128),
                            in_=zt[:, :a * D].rearrange("p (a d) -> p a d", a=a),
                        )
                        r0 += a * 128
                    else:
                        nc.sync.dma_start(out=part[r0:r0 + n, :], in_=zt[:n, :D])
                        r0 += n

            for part, nr in ((g1i_part, DI + W), (g1u_part, DU + W),
                             (g2u_part, DU + W), (g2i_part, DI + W),
                             (g3i_part, DI + W)):
                zero_part(part, nr)

            def spmm(table, part, nsec, nsb, idx_in, slot_in, w_in, sidx_in, tag):
                sub = int(os.environ.get("KERNEL_SUB", "3"))
                for s in range(nsec):
                    for isb in range(nsb):
                        g = s * nsb + isb  # global super-block index
                        idxt = sb.tile([128, IDX_SB // 16], i16, tag="idx")
                        nc.sync.dma_start(
                            out=idxt[:],
                            in_=idx_in[:, g * (IDX_SB // 16):(g + 1) * (IDX_SB // 16)])
                        slott = sb.tile([128, CH_SB], f32, tag="slot")
                        nc.sync.dma_start(
                            out=slott[:],
                            in_=slot_in[:, g * CH_SB:(g + 1) * CH_SB])
                        wt = sb.tile([128, CH_SB], f32, tag="w")
                        nc.sync.dma_start(
                            out=wt[:], in_=w_in[:, g * CH_SB:(g + 1) * CH_SB])
                        sidxt = sb.tile([128, ROWS_SB // 16], i16, tag="sidx")
                        nc.sync.dma_start(
                            out=sidxt[:],
                            in_=sidx_in[:, g * (ROWS_SB // 16):(g + 1) * (ROWS_SB // 16)])

                        gt = gp.tile([128, CH_SB * D], f32, tag="G")
                        nc.gpsimd.dma_gather(
                            out_ap=gt[:].rearrange("p (c d) -> p c d", c=CH_SB),
                            in_ap=table[s * SEC:(s + 1) * SEC, :],
                            idxs_ap=idxt[:],
                            num_idxs=IDX_SB,
                            num_idxs_reg=IDX_SB,
                            elem_size=D,
                            single_packet=False,
                        )
                        if sub == 0:
                            # gather only: consume G with a probe copy
                            nc.gpsimd.dma_start(out=part[0:128, :], in_=gt[:, 0:D])
                            continue
                        psA = pp.tile([64, 512], f32, tag="psA")
                        psB = pp.tile([64, 512], f32, tag="psB")
                        for blk in range(BPS):
                            ps = psA if blk < 8 else psB
                            col = blk % 8
                            for ch in range(CPB):
                                ci = blk * CPB + ch
                                st = spool.tile([128, W], f32, tag="S")
                                nc.vector.tensor_scalar(
                                    out=st[:],
                                    in0=iota_t[:],
                                    scalar1=slott[:, ci:ci + 1],
                                    scalar2=wt[:, ci:ci + 1],
                                    op0=mybir.AluOpType.is_equal,
                                    op1=mybir.AluOpType.mult,
                                )
                                nc.tensor.matmul(
                                    out=ps[:, col * D:(col + 1) * D],
                                    lhsT=st[:],
                                    rhs=gt[:, ci * D:(ci + 1) * D],
                                    start=(ch == 0),
                                    stop=(ch == CPB - 1),
                                )
                        stg = sb.tile([128, 512], f32, tag="stg")
                        nc.scalar.activation(
                            out=stg[0:64, :], in_=psA[:],
                            func=mybir.ActivationFunctionType.Copy)
                        nc.scalar.activation(
                            out=stg[64:128, :], in_=psB[:],
                            func=mybir.ActivationFunctionType.Copy)
                        if sub <= 1:
                            nc.gpsimd.dma_start(out=part[0:128, :], in_=stg[:, 0:D])
                            continue
                        nc.gpsimd.dma_scatter_add(
                            part[:],
                            stg[:].rearrange("p (c d) -> p c d", c=8),
                            sidxt[:],
                            ROWS_SB,
                            ROWS_SB,
                            D,
                        )

            stage = int(os.environ.get("KERNEL_STAGE", "0"))
            repeat = int(os.environ.get("KERNEL_REPEAT", "1"))
            for _rep in range(repeat):
                # hop 1
                spmm(t_eu, g1i_part, NSEC_IU, nsb_iu, iu_idx, iu_slot, iu_w, iu_sidx, "g1i")
                if stage != 1:
                    nc.gpsimd.collective_compute(
                        "AllGather", mybir.AluOpType.bypass, replica_groups=rg,
                        ins=[g1i_part[0:DI, :]], outs=[g1i_full[:]])
                if stage == 0:
                    spmm(t_ei, g1u_part, NSEC_UI, nsb_ui, ui_idx, ui_slot, ui_w, ui_sidx, "g1u")
                    nc.gpsimd.collective_compute(
                        "AllGather", mybir.AluOpType.bypass, replica_groups=rg,
                        ins=[g1u_part[0:DU, :]], outs=[g1u_full[:]])
                    # hop 2
                    spmm(g1i_full, g2u_part, NSEC_UI, nsb_ui, ui_idx, ui_slot, ui_w, ui_sidx, "g2u")
                    nc.gpsimd.collective_compute(
                        "AllGather", mybir.AluOpType.bypass, replica_groups=rg,
                        ins=[g2u_part[0:DU, :]], outs=[g2u_full[:]])
                    spmm(g1u_full, g2i_part, NSEC_IU, nsb_iu, iu_idx, iu_slot, iu_w, iu_sidx, "g2i")
                    # hop 3
                    spmm(g2u_full, g3i_part, NSEC_IU, nsb_iu, iu_idx, iu_slot, iu_w, iu_sidx, "g3i")

            # final: out = ei + 1/2 g1i + 1/3 g2i + 1/4 g3i  (local slices)
            def combine(r0, nrows):
                a = nrows // 128
                if a >= 1:
                    view = lambda t: t[r0:r0 + a * 128, :].rearrange(
                        "(a p) d -> p a d", p=128)
                    sview = lambda t: t[:, :a * D].rearrange("p (a d) -> p a d", a=a)
                    shape = [128, a * D]
                    pslice = (slice(0, 128), slice(0, a * D))
                else:
                    view = lambda t: t[r0:r0 + nrows, :]
                    sview = lambda t: t[:nrows, :D]
                    shape = [128, D]
                    pslice = (slice(0, nrows), slice(0, D))
                eit = sb.tile(shape, f32, tag="fin_e")
                g1t = sb.tile(shape, f32, tag="fin_1")
                g2t = sb.tile(shape, f32, tag="fin_2")
                g3t = sb.tile(shape, f32, tag="fin_3")
                acc = sb.tile(shape, f32, tag="fin_a")
                tmp = sb.tile(shape, f32, tag="fin_t")
                nc.sync.dma_start(out=sview(eit), in_=view(ei_slice))
                nc.sync.dma_start(out=sview(g1t), in_=view(g1i_part))
                nc.sync.dma_start(out=sview(g2t), in_=view(g2i_part))
                nc.sync.dma_start(out=sview(g3t), in_=view(g3i_part))
                nc.vector.tensor_scalar_mul(acc[pslice], g1t[pslice], 0.5)
                nc.vector.tensor_add(acc[pslice], acc[pslice], eit[pslice])
                nc.vector.tensor_scalar_mul(tmp[pslice], g2t[pslice], 1.0 / 3.0)
                nc.vector.tensor_add(acc[pslice], acc[pslice], tmp[pslice])
                nc.vector.tensor_scalar_mul(tmp[pslice], g3t[pslice], 0.25)
                nc.vector.tensor_add(acc[pslice], acc[pslice], tmp[pslice])
                if a >= 1:
                    nc.sync.dma_start(
                        out=out_ext[r0:r0 + a * 128, :].rearrange(
                            "(a p) d -> p a d", p=128),
                        in_=sview(acc))
                else:
                    nc.sync.dma_start(out=out_ext[r0:r0 + nrows, :], in_=sview(acc))

            full = (DI // (128 * 24)) * 128 * 24
            r0 = 0
            while r0 < DI:
                n = min(128 * 24, DI - r0)
                if n >= 128:
                    n = (n // 128) * 128
                combine(r0, n)
                r0 += n

    nc.compile()
    return nc


def kernel(embed_user, embed_item, edge_vals, u_idx, i_idx):
    global _LAST_RESULTS
    embed_user = np.asarray(embed_user, np.float32)
    embed_item = np.asarray(embed_item, np.float32)
    edge_vals = np.asarray(edge_vals, np.float32)
    u_idx = np.asarray(u_idx).astype(np.int64)
    i_idx = np.asarray(i_idx).astype(np.int64)

    # pack both SpMM edge types
    iu = _pack_type(i_idx, u_idx, edge_vals, DI, NSEC_IU)   # dest=item, src=user
    ui = _pack_type(u_idx, i_idx, edge_vals, DU, NSEC_UI)   # dest=user, src=item

    nc = _build_program(iu["nsb"], ui["nsb"])

    iota = np.broadcast_to(np.arange(W, dtype=np.float32), (128, W)).copy()
    in_maps = []
    for c in range(NCORES):
        in_maps.append({
            "embed_user": embed_user,
            "embed_item": embed_item,
            "ei_slice": np.ascontiguousarray(embed_item[c * DI:(c + 1) * DI]),
            "iota": iota,
            "iu_idx": iu["idx16"][c], "iu_slot": iu["slot"][c],
            "iu_w": iu["w"][c], "iu_sidx": iu["sidx16"][c],
            "ui_idx": ui["idx16"][c], "ui_slot": ui["slot"][c],
            "ui_w": ui["w"][c], "ui_sidx": ui["sidx16"][c],
        })

    trace = bool(int(os.environ.get("KERNEL_TRACE", "0")))
    res = bass_utils.run_bass_kernel_spmd(
        nc, in_maps, core_ids=list(range(NCORES)), trace=trace)
    _LAST_RESULTS = res
    out = np.concatenate([res.results[c]["out"] for c in range(NCORES)], axis=0)
    return out

